# revision 14
# baseline (speedup 1.0000x reference)
"""CapsuleLayer routing kernel for 8x Trainium2 NeuronCores (Bass/Tile).

Strategy
--------
Data-parallel over batch: core b handles x[b] (2048x512), W replicated.

The routing algebra is refactored so capsules (2048x32x64 per batch) are
never materialized:

    y      = r^T @ x                      [32, 512]   (it0: rows = sum_s x / 32)
    nodes  = diag_n(y @ W2)               [32, 64]    W2[i,(n,d)] = W[n,i,d]
    t      = tanh(nodes)
    u[n,:] = W[n] @ t[n]                  [32, 512]
    ag     = x @ u^T                      [2048, 32]
    b     += ag ; r = softmax_n(b)

All big contractions run on the PE at fp32r rate (1 cycle/row for moving
free dim >= 256). Host pre-arranges x^T, W2 and a pair-packed W layout so
every DMA is contiguous and no big on-chip transposes are needed.

Per-iteration device structure:
  nodes-dense: [32,2048] = yT^T @ W2  (diagonal blocks are the real nodes)
  selector:    mask*tanh(dense) chunks, then 16 PE transposes emit
               selx[128, j, 32]: chunk j's col m is nonzero only for
               m in {2j, 2j+1} where it equals [t[2j];0] / [0;t[2j+1]]
  u:           16 accumulating matmuls lhsT=selx[:,j,:], rhs=Wpair[:,j,:]
               -> exact row-aligned u [32,512] in one PSUM bank
               (cross rows are exactly zero), then 4 PE transposes -> uT
  ag:          agT[32,2048] = uT^T @ xT, block-transposed on DVE, then a
               strided DMA rearranges into b's [128s, 16, 32n] layout
  softmax:     exp (Act), segmented reduce + reciprocal + per-partition
               scale (DVE) -> r in y's lhsT layout
Final iteration: transposes of the unmasked dense go out as [128, 512];
the host picks the diagonal entries (pure indexing) during unshard.
"""

import sys

sys.path.insert(0, "/opt/trn_rl_repo")

import numpy as np

N_CORES = 8
S, I, N, D = 2048, 512, 32, 64
T = S // 128  # 16 s-tiles
KI = I // 128  # 4 i-tiles
NUM_ROUTING = 3
USE_F32R = True
AG_BF16 = True
OUT_BF16 = True

_CACHE = {}


def _build():
    import concourse.tile as tile
    from concourse import bacc, mybir
    from concourse.masks import make_identity

    f32 = mybir.dt.float32
    f32r = mybir.dt.float32r
    fr = f32r if USE_F32R else f32
    bf16 = mybir.dt.bfloat16
    ag_t = bf16 if AG_BF16 else fr
    out_t = bf16 if OUT_BF16 else fr
    Act = mybir.ActivationFunctionType
    Alu = mybir.AluOpType

    def mm(ap):
        return ap

    nc = bacc.Bacc("TRN2", target_bir_lowering=False, debug=False,
                   num_devices=N_CORES)
    x_d = nc.dram_tensor("x", [S, I], out_t, kind="ExternalInput").ap()
    xt_d = nc.dram_tensor("xt", [I, S], ag_t, kind="ExternalInput").ap()
    w2_d = nc.dram_tensor("w2", [I, N * D], out_t, kind="ExternalInput").ap()
    wp_d = nc.dram_tensor("wpair", [128, (N // 2) * I], ag_t,
                          kind="ExternalInput").ap()
    out_d = nc.dram_tensor("out", [N, N * D], f32,
                           kind="ExternalOutput").ap()

    with tile.TileContext(nc) as tc:
        with tc.tile_pool(name="sb", bufs=1) as sb, \
             tc.tile_pool(name="ps", bufs=1, space="PSUM") as ps:

            # ---- persistent SBUF tiles ----
            x_sb = sb.tile([128, T, I], out_t)
            xt_sb = sb.tile([128, KI, S], ag_t)
            w2_sb = sb.tile([128, KI, N * D], out_t)
            wp_sb = sb.tile([128, N // 2, I], ag_t)
            mask = sb.tile([N, N * D], f32)
            id32 = sb.tile([32, 32], f32)
            ones_inv = sb.tile([128, N], f32)
            b_sb = sb.tile([128, T, N], f32)
            ag_sb = sb.tile([128, T, N], f32)
            expb = sb.tile([128, T, N], f32)
            sums = sb.tile([128, T], f32)
            rec = sb.tile([128, T], f32)
            r_sb = sb.tile([128, T, N], out_t)
            sum0t = sb.tile([128, KI], f32)
            sum0rep = sb.tile([128, KI, N], out_t)

            # ---- input loads, ordered by first use on the chain:
            # xt (sum0 + ag) -> w2 (nodes0) -> wp (u0) -> x (y, it1) ----
            for k in range(KI):
                nc.sync.dma_start(out=xt_sb[:, k, :],
                                  in_=xt_d[128 * k:128 * (k + 1), :])
            for k in range(KI):
                nc.sync.dma_start(out=w2_sb[:, k, :],
                                  in_=w2_d[128 * k:128 * (k + 1), :])
            for k in range(KI):
                nc.sync.dma_start(out=wp_sb[:, 4 * k:4 * k + 4, :],
                                  in_=wp_d[:, 2048 * k:2048 * (k + 1)]
                                  .rearrange("p (j i) -> p j i", j=4))
            xv = x_d.rearrange("(t p) i -> p t i", p=128)
            for q in range(4):
                nc.sync.dma_start(out=x_sb[:, 4 * q:4 * q + 4, :],
                                  in_=xv[:, 4 * q:4 * q + 4, :])

            # ---- constants ----
            make_identity(nc, id32[:])
            nc.gpsimd.memset(mask[:], 0.0)
            # mask[n', (n,d)] = 1 iff n' == n :
            # affine = n' - n ; !=0 -> keep in_(0), else fill 1.0
            nc.gpsimd.affine_select(out=mask[:], in_=mask[:],
                                    compare_op=Alu.not_equal, fill=1.0,
                                    base=0, pattern=[[-1, N], [0, D]],
                                    channel_multiplier=1)
            nc.vector.memset(ones_inv[:], 1.0 / N)

            # ---- it0 lhsT: rows of sum_s(x)/32 replicated over n ----
            for k in range(KI):
                nc.vector.reduce_sum(out=sum0t[:, k:k + 1], in_=xt_sb[:, k, :],
                                     axis=mybir.AxisListType.X)
                nc.vector.tensor_scalar_mul(sum0rep[:, k, :], ones_inv[:],
                                            sum0t[:, k:k + 1])

            # ---------------- routine emitters ----------------
            def emit_nodes_dense(lhsT_of_k, it):
                chunks = []
                for c in range(4):
                    npc = ps.tile([128, 512], f32, tag=f"bank{c}",
                                  name=f"nodes_ps_{it}_{c}")
                    for k in range(KI):
                        nc.tensor.matmul(
                            npc[:N, :],
                            lhsT=mm(lhsT_of_k(k)),
                            rhs=mm(w2_sb[:, k, 512 * c:512 * (c + 1)]),
                            start=(k == 0), stop=(k == KI - 1))
                    chunks.append(npc)
                return chunks

            def emit_selx(nodes_ch, it, masked):
                """16 transposes of (mask*)tanh(dense) -> selx [128,(16,32)].

                Chunk j holds [t[2j];0] / [0;t[2j+1]] in cols 2j/2j+1 and
                zeros elsewhere (when masked). Final (unmasked) variant is
                only read back on the host, garbage cols ignored.
                """
                tm = sb.tile([N, N * D], f32, tag="tm", name=f"tm_{it}")
                selx_ps = ps.tile([128, T * N], f32, tag="selx_ps",
                                  name=f"selx_ps_{it}")
                selx_sb = sb.tile([128, T, N], ag_t, tag="selx_sb",
                                  name=f"selx_sb_{it}")
                for c in range(4):
                    cs = slice(512 * c, 512 * (c + 1))
                    if masked:
                        nc.vector.tensor_mul(tm[:, cs], nodes_ch[c][:N, :],
                                             mask[:, cs])
                    else:
                        nc.scalar.copy(tm[:, cs], nodes_ch[c][:N, :])
                    for jj in range(4):
                        j = 4 * c + jj
                        nc.tensor.matmul(selx_ps[:, 32 * j:32 * (j + 1)],
                                         lhsT=tm[:, 128 * j:128 * (j + 1)],
                                         rhs=id32[:], is_transpose=True,
                                         start=True, stop=True)
                    # tanh commutes with the transpose; fuse it into the copy
                    nc.scalar.activation(
                        selx_sb[:, 4 * c:4 * (c + 1), :],
                        selx_ps[:, 128 * c:128 * (c + 1)]
                        .rearrange("p (j m) -> p j m", m=N),
                        func=Act.Tanh)
                return selx_sb

            def emit_u(selx_sb, it):
                u_ps = ps.tile([N, I], f32, tag="small32", name=f"u_ps_{it}")
                for j in range(16):
                    nc.tensor.matmul(u_ps[:], lhsT=mm(selx_sb[:, j, :]),
                                     rhs=mm(wp_sb[:, j, :]),
                                     start=(j == 0), stop=(j == 15))
                u_sb = sb.tile([N, I], f32, tag="u_sb", name=f"u_sb_{it}")
                uT_ps = ps.tile([128, 128], f32, tag="t128",
                                name=f"uT_ps_{it}")
                uT_sb = sb.tile([128, 128], ag_t, tag="uT_sb",
                                name=f"uT_sb_{it}")
                for k in range(KI):
                    ks = slice(128 * k, 128 * (k + 1))
                    if k % 2 == 0:
                        nc.vector.tensor_copy(u_sb[:, ks], u_ps[:, ks])
                    else:
                        nc.scalar.copy(u_sb[:, ks], u_ps[:, ks])
                    nc.tensor.matmul(uT_ps[:, 32 * k:32 * (k + 1)],
                                     lhsT=u_sb[:, ks],
                                     rhs=id32[:], is_transpose=True,
                                     start=True, stop=True)
                    nc.scalar.copy(uT_sb[:, 32 * k:32 * (k + 1)],
                                   uT_ps[:, 32 * k:32 * (k + 1)])
                return uT_sb

            def emit_ag(uT_sb, it, first):
                # agT chunk -> SBUF (Act) -> 4 PE transposes landing in
                # b's [128s,(t,n)] layout, one PSUM tile per s-chunk so the
                # b-add / softmax / next-y chase chunk-by-chunk.
                for c in range(4):
                    cs = slice(512 * c, 512 * (c + 1))
                    agc = ps.tile([128, 512], f32, tag=f"bank{c}",
                                  name=f"ag_ps_{it}_{c}")
                    for k in range(KI):
                        nc.tensor.matmul(agc[:N, :],
                                         lhsT=mm(uT_sb[:, 32 * k:32 * (k + 1)]),
                                         rhs=mm(xt_sb[:, k, cs]),
                                         start=(k == 0), stop=(k == KI - 1))
                    agts = sb.tile([N, 512], f32, tag="agts", bufs=2,
                                   name=f"agts_{it}_{c}")
                    agbq = ps.tile([128, 4, N], f32,
                                   tag=["selx_ps", "t128", "bank1",
                                        "agb"][c],
                                   name=f"agb_ps_{it}_{c}")
                    for jj in range(4):
                        js = slice(128 * jj, 128 * (jj + 1))
                        if (c + jj) % 2 == 0:
                            nc.vector.tensor_copy(agts[:, js], agc[:N, js])
                        else:
                            nc.scalar.copy(agts[:, js], agc[:N, js])
                        nc.tensor.matmul(agbq[:, jj, :],
                                         lhsT=agts[:, js],
                                         rhs=id32[:], is_transpose=True,
                                         start=True, stop=True)
                    qs = slice(4 * c, 4 * (c + 1))
                    if first:
                        nc.vector.tensor_copy(b_sb[:, qs, :], agbq[:])
                    else:
                        nc.vector.tensor_add(b_sb[:, qs, :], b_sb[:, qs, :],
                                             agbq[:])

            def emit_softmax():
                for q in range(4):
                    qs = slice(4 * q, 4 * (q + 1))
                    nc.scalar.activation(expb[:, qs, :], b_sb[:, qs, :],
                                         func=Act.Exp)
                    nc.vector.reduce_sum(out=sums[:, qs], in_=expb[:, qs, :],
                                         axis=mybir.AxisListType.X)
                    nc.vector.reciprocal(rec[:, qs], sums[:, qs])
                    for t in range(4 * q, 4 * q + 4):
                        nc.vector.tensor_scalar_mul(r_sb[:, t, :],
                                                    expb[:, t, :],
                                                    rec[:, t:t + 1])

            def emit_y(it):
                y_ps = ps.tile([N, I], f32, tag="small32", name=f"y_ps_{it}")
                for t in range(T):
                    nc.tensor.matmul(y_ps[:], lhsT=mm(r_sb[:, t, :]),
                                     rhs=mm(x_sb[:, t, :]),
                                     start=(t == 0), stop=(t == T - 1))
                y_sb = sb.tile([N, I], f32, tag="y_sb", name=f"y_sb_{it}")
                yT_ps = ps.tile([128, 128], f32, tag="t128",
                                name=f"yT_ps_{it}")
                yT_sb = sb.tile([128, 128], out_t, tag="yT_sb",
                                name=f"yT_sb_{it}")
                for k in range(KI):
                    ks = slice(128 * k, 128 * (k + 1))
                    if k % 2 == 0:
                        nc.vector.tensor_copy(y_sb[:, ks], y_ps[:, ks])
                    else:
                        nc.scalar.copy(y_sb[:, ks], y_ps[:, ks])
                    nc.tensor.matmul(yT_ps[:, 32 * k:32 * (k + 1)],
                                     lhsT=y_sb[:, ks],
                                     rhs=id32[:], is_transpose=True,
                                     start=True, stop=True)
                    nc.scalar.copy(yT_sb[:, 32 * k:32 * (k + 1)],
                                   yT_ps[:, 32 * k:32 * (k + 1)])
                return yT_sb

            # ---------------- main schedule ----------------
            nodes_ch = emit_nodes_dense(lambda k: sum0rep[:, k, :], 0)
            selx = emit_selx(nodes_ch, 0, masked=True)
            uT = emit_u(selx, 0)
            emit_ag(uT, 0, first=True)

            for it in range(1, NUM_ROUTING):
                emit_softmax()
                yT = emit_y(it)
                nodes_ch = emit_nodes_dense(
                    lambda k, _y=yT: _y[:, 32 * k:32 * (k + 1)], it)
                if it < NUM_ROUTING - 1:
                    selx = emit_selx(nodes_ch, it, masked=True)
                    uT = emit_u(selx, it)
                    emit_ag(uT, it, first=False)

            # ---- final: copy dense chunks to SBUF, DMA out;
            # host picks nodes[n,d] = dense[n, 64n+d] during unshard ----
            tmf = sb.tile([N, N * D], f32, tag="tm", name="tm_final")
            for c in range(4):
                cs = slice(512 * c, 512 * (c + 1))
                if c % 2 == 0:
                    nc.vector.tensor_copy(tmf[:, cs], nodes_ch[c][:N, :])
                else:
                    nc.scalar.copy(tmf[:, cs], nodes_ch[c][:N, :])
                nc.sync.dma_start(out=out_d[:, cs], in_=tmf[:, cs])

    nc.compile()
    return nc


def _get_nc():
    if "nc" not in _CACHE:
        _CACHE["nc"] = _build()
    return _CACHE["nc"]


def extract_final(buf):
    """buf [32, 2048] dense rows -> nodes[n, d] = buf[n, 64n+d]."""
    buf = buf.reshape(N, N * D)
    n = np.arange(N)
    d = np.arange(D)
    return buf[n[:, None], (64 * n)[:, None] + d[None, :]]


def make_in_maps(x, W):
    x = np.asarray(x, dtype=np.float32)
    W = np.asarray(W, dtype=np.float32)
    ag_np = np.float32
    out_np = np.float32
    if AG_BF16 or OUT_BF16:
        import ml_dtypes
        if AG_BF16:
            ag_np = ml_dtypes.bfloat16
        if OUT_BF16:
            out_np = ml_dtypes.bfloat16
    w2 = np.ascontiguousarray(
        W.transpose(1, 0, 2).reshape(I, N * D).astype(out_np))
    # wpair[64h+d, 512j+i] = W[2j+h, i, d]
    wp = np.ascontiguousarray(
        W.reshape(N // 2, 2, I, D).transpose(1, 3, 0, 2).reshape(
            128, (N // 2) * I).astype(ag_np))
    maps = []
    for b in range(N_CORES):
        maps.append({
            "x": np.ascontiguousarray(x[b].astype(out_np)),
            "xt": np.ascontiguousarray(x[b].T.astype(ag_np)),
            "w2": w2,
            "wpair": wp,
        })
    return maps


def kernel(x, W):
    from concourse.bass_utils import run_bass_kernel_spmd

    nc = _get_nc()
    in_maps = make_in_maps(x, W)
    res = run_bass_kernel_spmd(nc, in_maps, list(range(N_CORES)))
    out = np.stack([extract_final(res.results[b]["out"])
                    for b in range(N_CORES)])
    return out.astype(np.float32)


# revision 15
# speedup vs baseline: 1.0374x; 1.0374x over previous
"""CapsuleLayer routing kernel for 8x Trainium2 NeuronCores (Bass/Tile).

Strategy
--------
Data-parallel over batch: core b handles x[b] (2048x512), W replicated.

The routing algebra is refactored so capsules (2048x32x64 per batch) are
never materialized:

    y      = r^T @ x                      [32, 512]   (it0: rows = sum_s x / 32)
    nodes  = diag_n(y @ W2)               [32, 64]    W2[i,(n,d)] = W[n,i,d]
    t      = tanh(nodes)
    u[n,:] = W[n] @ t[n]                  [32, 512]
    ag     = x @ u^T                      [2048, 32]
    b     += ag ; r = softmax_n(b)

All big contractions run on the PE at fp32r rate (1 cycle/row for moving
free dim >= 256). Host pre-arranges x^T, W2 and a pair-packed W layout so
every DMA is contiguous and no big on-chip transposes are needed.

Per-iteration device structure:
  nodes-dense: [32,2048] = yT^T @ W2  (diagonal blocks are the real nodes)
  selector:    mask*tanh(dense) chunks, then 16 PE transposes emit
               selx[128, j, 32]: chunk j's col m is nonzero only for
               m in {2j, 2j+1} where it equals [t[2j];0] / [0;t[2j+1]]
  u:           16 accumulating matmuls lhsT=selx[:,j,:], rhs=Wpair[:,j,:]
               -> exact row-aligned u [32,512] in one PSUM bank
               (cross rows are exactly zero), then 4 PE transposes -> uT
  ag:          agT[32,2048] = uT^T @ xT, block-transposed on DVE, then a
               strided DMA rearranges into b's [128s, 16, 32n] layout
  softmax:     exp (Act), segmented reduce + reciprocal + per-partition
               scale (DVE) -> r in y's lhsT layout
Final iteration: transposes of the unmasked dense go out as [128, 512];
the host picks the diagonal entries (pure indexing) during unshard.
"""

import sys

sys.path.insert(0, "/opt/trn_rl_repo")

import numpy as np

N_CORES = 8
S, I, N, D = 2048, 512, 32, 64
T = S // 128  # 16 s-tiles
KI = I // 128  # 4 i-tiles
NUM_ROUTING = 3
USE_F32R = True
AG_BF16 = True
OUT_BF16 = True

_CACHE = {}


def _build():
    import concourse.tile as tile
    from concourse import bacc, mybir
    from concourse.masks import make_identity

    f32 = mybir.dt.float32
    f32r = mybir.dt.float32r
    fr = f32r if USE_F32R else f32
    bf16 = mybir.dt.bfloat16
    ag_t = bf16 if AG_BF16 else fr
    out_t = bf16 if OUT_BF16 else fr
    Act = mybir.ActivationFunctionType
    Alu = mybir.AluOpType

    def mm(ap):
        return ap

    nc = bacc.Bacc("TRN2", target_bir_lowering=False, debug=False,
                   num_devices=N_CORES)
    x_d = nc.dram_tensor("x", [S, I], out_t, kind="ExternalInput").ap()
    xt_d = nc.dram_tensor("xt", [I, S], ag_t, kind="ExternalInput").ap()
    w2_d = nc.dram_tensor("w2", [I, N * D], out_t, kind="ExternalInput").ap()
    wp_d = nc.dram_tensor("wpair", [128, (N // 2) * I], ag_t,
                          kind="ExternalInput").ap()
    out_d = nc.dram_tensor("out", [N, N * D], f32,
                           kind="ExternalOutput").ap()

    with tile.TileContext(nc) as tc:
        with tc.tile_pool(name="sb", bufs=1) as sb, \
             tc.tile_pool(name="ps", bufs=1, space="PSUM") as ps:

            # ---- persistent SBUF tiles ----
            x_sb = sb.tile([128, T, I], out_t)
            xt_sb = sb.tile([128, KI, S], ag_t)
            w2_sb = sb.tile([128, KI, N * D], out_t)
            wp_sb = sb.tile([128, N // 2, I], ag_t)
            mask = sb.tile([N, N * D], f32)
            id32 = sb.tile([32, 32], f32)
            ones_inv = sb.tile([128, N], f32)
            b_sb = sb.tile([128, T, N], f32)
            ag_sb = sb.tile([128, T, N], f32)
            expb = sb.tile([128, T, N], f32)
            sums = sb.tile([128, T], f32)
            rec = sb.tile([128, T], f32)
            r_sb = sb.tile([128, T, N], out_t)
            sum0t = sb.tile([128, KI], f32)
            sum0rep = sb.tile([128, KI, N], out_t)

            # ---- input loads, ordered by first use on the chain:
            # xt (sum0 + ag) -> w2 (nodes0) -> wp (u0) -> x (y, it1) ----
            for k in range(KI):
                nc.sync.dma_start(out=xt_sb[:, k, :],
                                  in_=xt_d[128 * k:128 * (k + 1), :])
            for k in range(KI):
                nc.sync.dma_start(out=w2_sb[:, k, :],
                                  in_=w2_d[128 * k:128 * (k + 1), :])
            for k in range(KI):
                nc.sync.dma_start(out=wp_sb[:, 4 * k:4 * k + 4, :],
                                  in_=wp_d[:, 2048 * k:2048 * (k + 1)]
                                  .rearrange("p (j i) -> p j i", j=4))
            xv = x_d.rearrange("(t p) i -> p t i", p=128)
            for q in range(4):
                nc.sync.dma_start(out=x_sb[:, 4 * q:4 * q + 4, :],
                                  in_=xv[:, 4 * q:4 * q + 4, :])

            # ---- constants ----
            make_identity(nc, id32[:])
            nc.gpsimd.memset(mask[:], 0.0)
            # mask[n', (n,d)] = 1 iff n' == n :
            # affine = n' - n ; !=0 -> keep in_(0), else fill 1.0
            nc.gpsimd.affine_select(out=mask[:], in_=mask[:],
                                    compare_op=Alu.not_equal, fill=1.0,
                                    base=0, pattern=[[-1, N], [0, D]],
                                    channel_multiplier=1)
            nc.vector.memset(ones_inv[:], 1.0 / N)

            # ---- it0 lhsT: rows of sum_s(x)/32 replicated over n ----
            for k in range(KI):
                nc.vector.reduce_sum(out=sum0t[:, k:k + 1], in_=xt_sb[:, k, :],
                                     axis=mybir.AxisListType.X)
                nc.vector.tensor_scalar_mul(sum0rep[:, k, :], ones_inv[:],
                                            sum0t[:, k:k + 1])

            # ---------------- routine emitters ----------------
            def emit_nodes_dense(lhsT_of_k, it):
                chunks = []
                for c in range(4):
                    npc = ps.tile([128, 512], f32, tag=f"bank{c}",
                                  name=f"nodes_ps_{it}_{c}")
                    for k in range(KI):
                        nc.tensor.matmul(
                            npc[:N, :],
                            lhsT=mm(lhsT_of_k(k)),
                            rhs=mm(w2_sb[:, k, 512 * c:512 * (c + 1)]),
                            start=(k == 0), stop=(k == KI - 1))
                    chunks.append(npc)
                return chunks

            def emit_selx(nodes_ch, it, masked):
                """16 transposes of (mask*)tanh(dense) -> selx [128,(16,32)].

                Chunk j holds [t[2j];0] / [0;t[2j+1]] in cols 2j/2j+1 and
                zeros elsewhere (when masked). Final (unmasked) variant is
                only read back on the host, garbage cols ignored.
                """
                tm = sb.tile([N, N * D], f32, tag="tm", name=f"tm_{it}")
                selx_ps = ps.tile([128, T * N], f32, tag="selx_ps",
                                  name=f"selx_ps_{it}")
                selx_sb = sb.tile([128, T, N], ag_t, tag="selx_sb",
                                  name=f"selx_sb_{it}")
                for c in range(4):
                    cs = slice(512 * c, 512 * (c + 1))
                    if masked:
                        nc.vector.tensor_mul(tm[:, cs], nodes_ch[c][:N, :],
                                             mask[:, cs])
                    else:
                        nc.scalar.copy(tm[:, cs], nodes_ch[c][:N, :])
                    for jj in range(4):
                        j = 4 * c + jj
                        nc.tensor.matmul(selx_ps[:, 32 * j:32 * (j + 1)],
                                         lhsT=tm[:, 128 * j:128 * (j + 1)],
                                         rhs=id32[:], is_transpose=True,
                                         start=True, stop=True)
                    # tanh commutes with the transpose; fuse it into the copy
                    nc.scalar.activation(
                        selx_sb[:, 4 * c:4 * (c + 1), :],
                        selx_ps[:, 128 * c:128 * (c + 1)]
                        .rearrange("p (j m) -> p j m", m=N),
                        func=Act.Tanh)
                return selx_sb

            def emit_u(selx_sb, it):
                u_ps = ps.tile([N, I], f32, tag="small32", name=f"u_ps_{it}")
                for j in range(16):
                    nc.tensor.matmul(u_ps[:], lhsT=mm(selx_sb[:, j, :]),
                                     rhs=mm(wp_sb[:, j, :]),
                                     start=(j == 0), stop=(j == 15))
                u_sb = sb.tile([N, I], f32, tag="u_sb", name=f"u_sb_{it}")
                uT_ps = ps.tile([128, 128], f32, tag="t128",
                                name=f"uT_ps_{it}")
                uT_sb = sb.tile([128, 128], ag_t, tag="uT_sb",
                                name=f"uT_sb_{it}")
                for k in range(KI):
                    ks = slice(128 * k, 128 * (k + 1))
                    if k % 2 == 0:
                        nc.vector.tensor_copy(u_sb[:, ks], u_ps[:, ks])
                    else:
                        nc.scalar.copy(u_sb[:, ks], u_ps[:, ks])
                    nc.tensor.matmul(uT_ps[:, 32 * k:32 * (k + 1)],
                                     lhsT=u_sb[:, ks],
                                     rhs=id32[:], is_transpose=True,
                                     start=True, stop=True)
                    nc.scalar.copy(uT_sb[:, 32 * k:32 * (k + 1)],
                                   uT_ps[:, 32 * k:32 * (k + 1)])
                return uT_sb

            def emit_ag(uT_sb, it, first):
                # agT chunk -> SBUF (Act) -> 4 PE transposes landing in
                # b's [128s,(t,n)] layout, one PSUM tile per s-chunk so the
                # b-add / softmax / next-y chase chunk-by-chunk.
                for c in range(4):
                    cs = slice(512 * c, 512 * (c + 1))
                    agc = ps.tile([128, 512], f32, tag=f"bank{c}",
                                  name=f"ag_ps_{it}_{c}")
                    for k in range(KI):
                        nc.tensor.matmul(agc[:N, :],
                                         lhsT=mm(uT_sb[:, 32 * k:32 * (k + 1)]),
                                         rhs=mm(xt_sb[:, k, cs]),
                                         start=(k == 0), stop=(k == KI - 1))
                    agts = sb.tile([N, 512], f32, tag="agts", bufs=2,
                                   name=f"agts_{it}_{c}")
                    if c % 2 == 0:
                        nc.vector.tensor_copy(agts[:], agc[:N, :])
                    else:
                        nc.scalar.copy(agts[:], agc[:N, :])
                    agbq = ps.tile([128, 4, N], f32,
                                   tag=["selx_ps", "t128", "bank1",
                                        "agb"][c],
                                   name=f"agb_ps_{it}_{c}")
                    for jj in range(4):
                        nc.tensor.matmul(agbq[:, jj, :],
                                         lhsT=agts[:, 128 * jj:128 * (jj + 1)],
                                         rhs=id32[:], is_transpose=True,
                                         start=True, stop=True)
                    qs = slice(4 * c, 4 * (c + 1))
                    if first:
                        nc.vector.tensor_copy(b_sb[:, qs, :], agbq[:])
                    else:
                        nc.vector.tensor_add(b_sb[:, qs, :], b_sb[:, qs, :],
                                             agbq[:])

            def emit_softmax():
                for q in range(4):
                    qs = slice(4 * q, 4 * (q + 1))
                    nc.scalar.activation(expb[:, qs, :], b_sb[:, qs, :],
                                         func=Act.Exp)
                    nc.vector.reduce_sum(out=sums[:, qs], in_=expb[:, qs, :],
                                         axis=mybir.AxisListType.X)
                    nc.vector.reciprocal(rec[:, qs], sums[:, qs])
                    for t in range(4 * q, 4 * q + 4):
                        nc.vector.tensor_scalar_mul(r_sb[:, t, :],
                                                    expb[:, t, :],
                                                    rec[:, t:t + 1])

            def emit_y(it):
                y_ps = ps.tile([N, I], f32, tag="small32", name=f"y_ps_{it}")
                for t in range(T):
                    nc.tensor.matmul(y_ps[:], lhsT=mm(r_sb[:, t, :]),
                                     rhs=mm(x_sb[:, t, :]),
                                     start=(t == 0), stop=(t == T - 1))
                y_sb = sb.tile([N, I], f32, tag="y_sb", name=f"y_sb_{it}")
                yT_ps = ps.tile([128, 128], f32, tag="t128",
                                name=f"yT_ps_{it}")
                yT_sb = sb.tile([128, 128], out_t, tag="yT_sb",
                                name=f"yT_sb_{it}")
                for k in range(KI):
                    ks = slice(128 * k, 128 * (k + 1))
                    if k % 2 == 0:
                        nc.vector.tensor_copy(y_sb[:, ks], y_ps[:, ks])
                    else:
                        nc.scalar.copy(y_sb[:, ks], y_ps[:, ks])
                    nc.tensor.matmul(yT_ps[:, 32 * k:32 * (k + 1)],
                                     lhsT=y_sb[:, ks],
                                     rhs=id32[:], is_transpose=True,
                                     start=True, stop=True)
                    nc.scalar.copy(yT_sb[:, 32 * k:32 * (k + 1)],
                                   yT_ps[:, 32 * k:32 * (k + 1)])
                return yT_sb

            # ---------------- main schedule ----------------
            nodes_ch = emit_nodes_dense(lambda k: sum0rep[:, k, :], 0)
            selx = emit_selx(nodes_ch, 0, masked=True)
            uT = emit_u(selx, 0)
            emit_ag(uT, 0, first=True)

            for it in range(1, NUM_ROUTING):
                emit_softmax()
                yT = emit_y(it)
                nodes_ch = emit_nodes_dense(
                    lambda k, _y=yT: _y[:, 32 * k:32 * (k + 1)], it)
                if it < NUM_ROUTING - 1:
                    selx = emit_selx(nodes_ch, it, masked=True)
                    uT = emit_u(selx, it)
                    emit_ag(uT, it, first=False)

            # ---- final: copy dense chunks to SBUF, DMA out;
            # host picks nodes[n,d] = dense[n, 64n+d] during unshard ----
            tmf = sb.tile([N, N * D], f32, tag="tm", name="tm_final")
            for c in range(4):
                cs = slice(512 * c, 512 * (c + 1))
                if c % 2 == 0:
                    nc.vector.tensor_copy(tmf[:, cs], nodes_ch[c][:N, :])
                else:
                    nc.scalar.copy(tmf[:, cs], nodes_ch[c][:N, :])
                nc.sync.dma_start(out=out_d[:, cs], in_=tmf[:, cs])

    nc.compile()
    return nc


def _get_nc():
    if "nc" not in _CACHE:
        _CACHE["nc"] = _build()
    return _CACHE["nc"]


def extract_final(buf):
    """buf [32, 2048] dense rows -> nodes[n, d] = buf[n, 64n+d]."""
    buf = buf.reshape(N, N * D)
    n = np.arange(N)
    d = np.arange(D)
    return buf[n[:, None], (64 * n)[:, None] + d[None, :]]


def make_in_maps(x, W):
    x = np.asarray(x, dtype=np.float32)
    W = np.asarray(W, dtype=np.float32)
    ag_np = np.float32
    out_np = np.float32
    if AG_BF16 or OUT_BF16:
        import ml_dtypes
        if AG_BF16:
            ag_np = ml_dtypes.bfloat16
        if OUT_BF16:
            out_np = ml_dtypes.bfloat16
    w2 = np.ascontiguousarray(
        W.transpose(1, 0, 2).reshape(I, N * D).astype(out_np))
    # wpair[64h+d, 512j+i] = W[2j+h, i, d]
    wp = np.ascontiguousarray(
        W.reshape(N // 2, 2, I, D).transpose(1, 3, 0, 2).reshape(
            128, (N // 2) * I).astype(ag_np))
    maps = []
    for b in range(N_CORES):
        maps.append({
            "x": np.ascontiguousarray(x[b].astype(out_np)),
            "xt": np.ascontiguousarray(x[b].T.astype(ag_np)),
            "w2": w2,
            "wpair": wp,
        })
    return maps


def kernel(x, W):
    from concourse.bass_utils import run_bass_kernel_spmd

    nc = _get_nc()
    in_maps = make_in_maps(x, W)
    res = run_bass_kernel_spmd(nc, in_maps, list(range(N_CORES)))
    out = np.stack([extract_final(res.results[b]["out"])
                    for b in range(N_CORES)])
    return out.astype(np.float32)
